# revision 11
# baseline (speedup 1.0000x reference)
"""HardBinaryVote Trainium2 kernel.

out[s] = (sum_m w[m]*votes[m,s] > sum_m w[m]/2)  as int32, votes in {0,1}.

Strategy (8 NeuronCores, sample-sharded, mixed fp16 x fp8):
  - Host shards samples 8-ways and re-codes each 0/1 int32 vote as the fp8
    e4m3 byte for 0.0/1.0 (0x00/0x38) during sharding: HBM traffic drops 4x
    vs int32 (15.75 MB/core). Two samples stack on the partition axis (2
    blocks of 63 models = 126 rows), so each rhs column carries 2 samples.
  - PE: plain-mode matmuls with fp16 weights (lhsT) against the fp8 votes
    (rhs) - the PE computes this mixed-dtype product at full fp16 weight
    precision (verified on HW), so a SINGLE pass gives ~11-bit weights
    (139/2M flipped decisions, rel err 0.012). 1 column/cycle, 2
    samples/column.
  - Matmul outputs must land at PSUM partition base 0, so 32 consecutive
    512-column slots chain into one [64, 512] PSUM bank via accumulation
    passes with shifting 2-column weight windows (slot s -> partitions
    [2j, 2j+2), j = s mod 32). One DVE is_gt per 32-slot supergroup
    thresholds 32768 samples in a single [64, 512] instruction.
  - Votes are zero-padded host-side to a whole number of 512-col slots;
    output int32 [64, 8, 512] per core; host inverts the (j, r, sg, col)
    interleave with one reshape/transpose and trims the padding.
"""

import os as _os
import sys

import numpy as np

sys.path.insert(0, "/opt/trn_rl_repo")

import ml_dtypes  # noqa: E402

from concourse import bacc, bass_utils, mybir, tile  # noqa: E402

N_MODELS = 63
N_SAMPLES = 2_000_000
N_CORES = 8
S_CORE = N_SAMPLES // N_CORES  # 250000 samples per core
KP = 2 * N_MODELS  # 126 partition rows

COLS = 512  # columns per matmul slot (one PSUM bank); 2 samples per column
WIN = 32  # chained window passes per supergroup -> M=64 out partitions
N_SLOTS = (S_CORE // 2 + COLS - 1) // COLS  # 245 slots (last padded)
G_PAD = N_SLOTS * COLS  # 125440 padded columns per core
NSG = (N_SLOTS + WIN - 1) // WIN  # 8 supergroups: 32*7 + 21
TILE_SLOTS = int(_os.environ.get("K_TILE_SLOTS", "4"))  # slots per input DMA
V_BUFS = int(_os.environ.get("K_V_BUFS", "16"))
PS_BUFS = int(_os.environ.get("K_PS_BUFS", "4"))
N_QUEUES = int(_os.environ.get("K_QUEUES", "3"))

E4 = ml_dtypes.float8_e4m3
FP8_ONE = 0x38  # e4m3 bit pattern of 1.0

_last_results = None  # BassKernelResults of the most recent run (for test.py)


def _build_program(threshold: float):
    nc = bacc.Bacc("TRN2", target_bir_lowering=False, debug=False)

    votes_d = nc.dram_tensor(
        "votes", [KP, G_PAD], mybir.dt.float8e4, kind="ExternalInput"
    )
    wp_d = nc.dram_tensor(
        "wp", [KP, WIN, 64], mybir.dt.float16, kind="ExternalInput"
    )
    out_d = nc.dram_tensor(
        "out", [64, NSG, 512], mybir.dt.int8, kind="ExternalOutput"
    )

    with tile.TileContext(nc) as tc:
        with (
            tc.tile_pool(name="w", bufs=1) as wpool,
            tc.tile_pool(name="v", bufs=V_BUFS) as vpool,
            tc.tile_pool(name="o", bufs=2) as opool,
            tc.tile_pool(name="ps", bufs=PS_BUFS, space="PSUM") as ppool,
        ):
            wt = wpool.tile([KP, WIN, 64], mybir.dt.float16, tag="wp")
            # window 0 first so the first matmul isn't gated on all weights
            nc.sync.dma_start(out=wt[:, 0], in_=wp_d[:, 0])
            nc.sync.dma_start(out=wt[:, 1:], in_=wp_d[:, 1:])

            queues = [nc.gpsimd, nc.scalar, nc.sync][:N_QUEUES]
            n_dma = 0

            for g in range(NSG):
                slots = min(WIN, N_SLOTS - g * WIN)
                gc0 = g * WIN * COLS

                vts = []  # (slot offset, n slots, tile)
                off = 0
                while off < slots:
                    ds = min(TILE_SLOTS, slots - off)
                    vt = vpool.tile([KP, ds * COLS], mybir.dt.float8e4)
                    # sync (queue 2) also carries the weights DMA: give it two
                    # fewer input tiles so all three queues drain together
                    if N_QUEUES >= 3 and n_dma < 4:
                        q = queues[n_dma % 2]
                    else:
                        q = queues[n_dma % N_QUEUES]
                    q.dma_start(
                        out=vt[:],
                        in_=votes_d[:, gc0 + off * COLS : gc0 + (off + ds) * COLS],
                    )
                    n_dma += 1
                    vts.append((off, ds, vt))
                    off += ds

                def rhs_of(s):
                    for toff, ds, vt in vts:
                        if toff <= s < toff + ds:
                            c0 = (s - toff) * COLS
                            return vt[:, c0 : c0 + COLS]
                    raise AssertionError(s)

                ps = ppool.tile([64, 512], mybir.dt.float32)
                # slot s -> window j=s: out partitions [2j, 2j+2)
                for j in range(slots):
                    nc.tensor.matmul(
                        ps[:],
                        wt[:, j],
                        rhs_of(j),
                        start=(j == 0),
                        stop=(j == slots - 1),
                    )

                ot = opool.tile([64, 512], mybir.dt.int8)
                nc.vector.tensor_scalar(
                    out=ot[:],
                    in0=ps[:],
                    scalar1=float(threshold),
                    scalar2=None,
                    op0=mybir.AluOpType.is_gt,
                )
                nc.sync.dma_start(out=out_d[:, g, :], in_=ot[:])

    nc.compile()
    return nc


def _weight_windows(w16: np.ndarray) -> np.ndarray:
    """Window j holds the fp16 weights in columns 2j (block 0) / 2j+1 (blk 1)."""
    wp = np.zeros((KP, WIN, 64), dtype=np.float16)
    for j in range(WIN):
        wp[:N_MODELS, j, 2 * j] = w16
        wp[N_MODELS:, j, 2 * j + 1] = w16
    return wp


def kernel(votes: np.ndarray, vote_weights: np.ndarray) -> np.ndarray:
    global _last_results
    votes = np.ascontiguousarray(votes, dtype=np.int32)
    w = np.asarray(vote_weights, dtype=np.float32)
    assert votes.shape == (N_MODELS, N_SAMPLES)

    threshold = float(np.float32(w.astype(np.float64).sum() / 2.0))
    wp = _weight_windows(w.astype(np.float16))

    # 0/1 -> fp8 e4m3 bytes for 0.0/1.0
    V8 = np.where(votes != 0, np.uint8(FP8_ONE), np.uint8(0))

    in_maps = []
    for c in range(N_CORES):
        sh = V8[:, c * S_CORE : (c + 1) * S_CORE].reshape(N_MODELS, S_CORE // 2, 2)
        dev = np.zeros((KP, G_PAD), np.uint8)
        dev[:N_MODELS, : S_CORE // 2] = sh[:, :, 0]  # r=0: sample 2C
        dev[N_MODELS:, : S_CORE // 2] = sh[:, :, 1]  # r=1: sample 2C+1
        in_maps.append({"votes": dev.view(E4), "wp": wp})

    nc = _build_program(threshold)
    res = bass_utils.run_bass_kernel_spmd(nc, in_maps, core_ids=list(range(N_CORES)))
    _last_results = res

    outs = []
    for c in range(N_CORES):
        O = res.results[c]["out"]  # [64, NSG, 512] int8
        # partition p = 2j + r; sample = 32768*g + 1024*j + 2*n + r
        O4 = O.reshape(WIN, 2, NSG, 512)  # [j, r, g, n]
        outs.append(O4.transpose(2, 0, 3, 1).reshape(-1)[:S_CORE])
    return np.ascontiguousarray(np.concatenate(outs).astype(np.int32))


# revision 14
# speedup vs baseline: 1.0242x; 1.0242x over previous
"""HardBinaryVote Trainium2 kernel.

out[s] = (sum_m w[m]*votes[m,s] > sum_m w[m]/2)  as int32, votes in {0,1}.

Strategy (8 NeuronCores, sample-sharded, mixed fp16 x fp8):
  - Host shards samples 8-ways and re-codes each 0/1 int32 vote as the fp8
    e4m3 byte for 0.0/1.0 (0x00/0x38) during sharding: HBM traffic drops 4x
    vs int32 (15.75 MB/core). Two samples stack on the partition axis (2
    blocks of 63 models = 126 rows), so each rhs column carries 2 samples.
  - PE: plain-mode matmuls with fp16 weights (lhsT) against the fp8 votes
    (rhs) - the PE computes this mixed-dtype product at full fp16 weight
    precision (verified on HW), so a SINGLE pass gives ~11-bit weights
    (139/2M flipped decisions, rel err 0.012). 1 column/cycle, 2
    samples/column.
  - Matmul outputs must land at PSUM partition base 0, so 32 consecutive
    512-column slots chain into one [64, 512] PSUM bank via accumulation
    passes with shifting 2-column weight windows (slot s -> partitions
    [2j, 2j+2), j = s mod 32). One DVE is_gt per 32-slot supergroup
    thresholds 32768 samples in a single [64, 512] instruction.
  - Votes are zero-padded host-side to a whole number of 512-col slots;
    output int32 [64, 8, 512] per core; host inverts the (j, r, sg, col)
    interleave with one reshape/transpose and trims the padding.
"""

import os as _os
import sys

import numpy as np

sys.path.insert(0, "/opt/trn_rl_repo")

import ml_dtypes  # noqa: E402

from concourse import bacc, bass_utils, mybir, tile  # noqa: E402

N_MODELS = 63
N_SAMPLES = 2_000_000
N_CORES = 8
S_CORE = N_SAMPLES // N_CORES  # 250000 samples per core
KP = 2 * N_MODELS  # 126 partition rows

COLS = 512  # columns per matmul slot (one PSUM bank); 2 samples per column
WIN = 32  # chained window passes per supergroup -> M=64 out partitions
N_SLOTS = (S_CORE // 2 + COLS - 1) // COLS  # 245 slots (last padded)
G_PAD = N_SLOTS * COLS  # 125440 padded columns per core
NSG = (N_SLOTS + WIN - 1) // WIN  # 8 supergroups: 32*7 + 21
TILE_SLOTS = int(_os.environ.get("K_TILE_SLOTS", "4"))  # slots per input DMA
V_BUFS = int(_os.environ.get("K_V_BUFS", "16"))
PS_BUFS = int(_os.environ.get("K_PS_BUFS", "4"))
N_QUEUES = int(_os.environ.get("K_QUEUES", "3"))

E4 = ml_dtypes.float8_e4m3
FP8_ONE = 0x38  # e4m3 bit pattern of 1.0

_last_results = None  # BassKernelResults of the most recent run (for test.py)


def _build_program(threshold: float):
    nc = bacc.Bacc("TRN2", target_bir_lowering=False, debug=False)

    votes_d = nc.dram_tensor(
        "votes", [KP, G_PAD], mybir.dt.float8e4, kind="ExternalInput"
    )
    wp_d = nc.dram_tensor(
        "wp", [KP, WIN, 64], mybir.dt.float16, kind="ExternalInput"
    )
    out_d = nc.dram_tensor(
        "out", [64, NSG, 512], mybir.dt.int8, kind="ExternalOutput"
    )

    with tile.TileContext(nc) as tc:
        with (
            tc.tile_pool(name="w", bufs=1) as wpool,
            tc.tile_pool(name="v", bufs=V_BUFS) as vpool,
            tc.tile_pool(name="o", bufs=2) as opool,
            tc.tile_pool(name="ps", bufs=PS_BUFS, space="PSUM") as ppool,
        ):
            wt = wpool.tile([KP, WIN, 64], mybir.dt.float16, tag="wp")
            # window 0 first so the first matmul isn't gated on all weights
            nc.sync.dma_start(out=wt[:, 0], in_=wp_d[:, 0])
            nc.sync.dma_start(out=wt[:, 1:], in_=wp_d[:, 1:])

            queues = [nc.gpsimd, nc.scalar, nc.sync][:N_QUEUES]
            # Measured per-queue DMA rates differ (gpsimd SWDGE ~100 GB/s,
            # scalar ~88, sync ~79 minus the weights/output DMAs it also
            # carries): weight the input-tile split so all queues drain
            # together.
            q_weight = [100.0, 88.0, 66.0][:N_QUEUES]
            q_load = [0.0] * N_QUEUES
            if N_QUEUES >= 3:
                q_load[2] = 8.0  # weights preload rides the sync queue

            for g in range(NSG):
                slots = min(WIN, N_SLOTS - g * WIN)
                gc0 = g * WIN * COLS

                vts = []  # (slot offset, n slots, tile)
                off = 0
                while off < slots:
                    ds = min(TILE_SLOTS, slots - off)
                    vt = vpool.tile([KP, ds * COLS], mybir.dt.float8e4)
                    qi = min(range(N_QUEUES), key=lambda i: q_load[i] / q_weight[i])
                    q_load[qi] += ds
                    queues[qi].dma_start(
                        out=vt[:],
                        in_=votes_d[:, gc0 + off * COLS : gc0 + (off + ds) * COLS],
                    )
                    vts.append((off, ds, vt))
                    off += ds

                def rhs_of(s):
                    for toff, ds, vt in vts:
                        if toff <= s < toff + ds:
                            c0 = (s - toff) * COLS
                            return vt[:, c0 : c0 + COLS]
                    raise AssertionError(s)

                ps = ppool.tile([64, 512], mybir.dt.float32)
                # slot s -> window j=s: out partitions [2j, 2j+2)
                for j in range(slots):
                    nc.tensor.matmul(
                        ps[:],
                        wt[:, j],
                        rhs_of(j),
                        start=(j == 0),
                        stop=(j == slots - 1),
                    )

                ot = opool.tile([64, 512], mybir.dt.int8)
                nc.vector.tensor_scalar(
                    out=ot[:],
                    in0=ps[:],
                    scalar1=float(threshold),
                    scalar2=None,
                    op0=mybir.AluOpType.is_gt,
                )
                nc.sync.dma_start(out=out_d[:, g, :], in_=ot[:])

    nc.compile()
    return nc


def _weight_windows(w16: np.ndarray) -> np.ndarray:
    """Window j holds the fp16 weights in columns 2j (block 0) / 2j+1 (blk 1)."""
    wp = np.zeros((KP, WIN, 64), dtype=np.float16)
    for j in range(WIN):
        wp[:N_MODELS, j, 2 * j] = w16
        wp[N_MODELS:, j, 2 * j + 1] = w16
    return wp


def kernel(votes: np.ndarray, vote_weights: np.ndarray) -> np.ndarray:
    global _last_results
    votes = np.ascontiguousarray(votes, dtype=np.int32)
    w = np.asarray(vote_weights, dtype=np.float32)
    assert votes.shape == (N_MODELS, N_SAMPLES)

    threshold = float(np.float32(w.astype(np.float64).sum() / 2.0))
    wp = _weight_windows(w.astype(np.float16))

    # 0/1 -> fp8 e4m3 bytes for 0.0/1.0
    V8 = np.where(votes != 0, np.uint8(FP8_ONE), np.uint8(0))

    in_maps = []
    for c in range(N_CORES):
        sh = V8[:, c * S_CORE : (c + 1) * S_CORE].reshape(N_MODELS, S_CORE // 2, 2)
        dev = np.zeros((KP, G_PAD), np.uint8)
        dev[:N_MODELS, : S_CORE // 2] = sh[:, :, 0]  # r=0: sample 2C
        dev[N_MODELS:, : S_CORE // 2] = sh[:, :, 1]  # r=1: sample 2C+1
        in_maps.append({"votes": dev.view(E4), "wp": wp})

    nc = _build_program(threshold)
    res = bass_utils.run_bass_kernel_spmd(nc, in_maps, core_ids=list(range(N_CORES)))
    _last_results = res

    outs = []
    for c in range(N_CORES):
        O = res.results[c]["out"]  # [64, NSG, 512] int8
        # partition p = 2j + r; sample = 32768*g + 1024*j + 2*n + r
        O4 = O.reshape(WIN, 2, NSG, 512)  # [j, r, g, n]
        outs.append(O4.transpose(2, 0, 3, 1).reshape(-1)[:S_CORE])
    return np.ascontiguousarray(np.concatenate(outs).astype(np.int32))
